# revision 4
# baseline (speedup 1.0000x reference)
"""Trainium2 Bass kernel for nn_CleanAttention (sliding-window GQA attention).

Problem: x[2,4096,2048] -> qkv proj -> rope -> sliding-window (256) attention
(16 q heads, 4 kv heads, d=128) -> o proj.

Sharding: 8 cores = batch(2) x token-quarters(4). Each core computes all 16
heads for its 1024 tokens, using a 256-token key/value halo on the left.
Outputs concatenate: no inter-core reduction.

All matmul operands are bf16 (psum accumulation f32): same PE rate as f32r
but half the DMA traffic, half the DVE cost, half the SBUF footprint.

Dataflow (per core):
  phase B: V = x @ wv.T; x/wv streamed per-kc so the first matmul starts
           ~1.5us in (kc-outer over 8 psum banks for token tiles 0..7).
           K^T = wk_p @ x.T + rope (4 groups).
  phase C: Q^T per head over all 1024 own tokens (wq streamed once) + rope
  phase D: per chunk c (128 queries):
     per kv-group g: S^T[kb] = K^T_blk @ Q^T (3 psum banks)
                     E = exp(scale*S^T) (ACT -> bf16), E *= 0/1 masks (DVE)
                     sums = ones @ E; outT = V_blk @ E (psum)
                     yt = outT * (1/sums) (DVE -> bf16)
     interleaved with the full O-projection of chunk c-1 (wo fully resident,
     quarter per g) as PE bubble-filler while exp/mask are in flight.

RoPE: wq/wk rows host-permuted per head to [even dims | odd dims]. With
cosdup = [cos|cos] and sinsig = [+sin|-sin] lookup tiles:
  A = raw * cosdup;  B[0:64] = raw[64:128]*sinsig[64:128] (= -o*sin),
  B[64:128] = raw[0:64]*sinsig[0:64] (= +e*sin);  out = A + B
i.e. 4 DVE ops per tile, all bf16 (2x DVE mode).
"""

import math

import numpy as np

import concourse.bass as bass
import concourse.mybir as mybir
import concourse.tile as tile
from concourse import bacc
from concourse import bass_utils

B, T, C = 2, 4096, 2048
NH, NKV, D = 16, 4, 128
WINDOW = 256
N_CORES = 8
TCORE = 1024  # own tokens per core
HALO = 256
TX = TCORE + HALO  # 1280
NG = 4  # kv groups
GH = 4  # q heads per group
NCHUNK = 8  # query chunks of 128 per core
SCALE = 1.0 / math.sqrt(D)

f32 = mybir.dt.float32
bf16 = mybir.dt.bfloat16

_CACHE = {}


def _build_nc(repeat=1):
    nc = bacc.Bacc("TRN2", target_bir_lowering=False, debug=False)

    xt = nc.dram_tensor("xt", [16, 128, TX], bf16, kind="ExternalInput")
    wq_t = nc.dram_tensor("wq_t", [NH, 128, 16, 128], bf16, kind="ExternalInput")
    wk_t = nc.dram_tensor("wk_t", [NKV, 128, 16, 128], bf16, kind="ExternalInput")
    wv_t = nc.dram_tensor("wv_t", [16, 128, 512], bf16, kind="ExternalInput")
    wo01_t = nc.dram_tensor("wo01_t", [128, 16, 2, 512], bf16, kind="ExternalInput")
    wo23_t = nc.dram_tensor("wo23_t", [128, 16, 2, 512], bf16, kind="ExternalInput")
    cs_t = nc.dram_tensor("cs_t", [2, 128, TX], bf16, kind="ExternalInput")
    masks = nc.dram_tensor("masks", [3, 128, 1024], bf16, kind="ExternalInput")
    causal = nc.dram_tensor("causal", [128, 512], bf16, kind="ExternalInput")
    ones_in = nc.dram_tensor("ones_in", [128, 128], bf16, kind="ExternalInput")
    o_out = nc.dram_tensor("o_out", [TCORE, C], f32, kind="ExternalOutput")

    exp_t = mybir.ActivationFunctionType.Exp

    with tile.TileContext(nc) as tc:
        with tc.sbuf_pool(name="fixed", bufs=1) as fixed:
            cos_sb = fixed.tile([128, TX], bf16)
            sin_sb = fixed.tile([128, TX], bf16)
            ones_sb = fixed.tile([128, 128], bf16)
            causal_sb = fixed.tile([128, 512], bf16)
            m01_sb = [
                fixed.tile([128, 1024], bf16, name=f"m01_{i}", tag=f"m01_{i}")
                for i in range(3)
            ]

            for rep in range(repeat):
                with (
                    tc.sbuf_pool(name="ktp", bufs=1) as ktp,
                    tc.sbuf_pool(name="vp", bufs=1) as vp,
                    tc.sbuf_pool(name="qtp", bufs=1) as qtp,
                    tc.sbuf_pool(name="wo01p", bufs=1) as wo01p,
                ):
                    kt_g = [
                        ktp.tile([128, TX], bf16, name=f"ktg{g}", tag=f"ktg{g}")
                        for g in range(NG)
                    ]
                    v_t = [
                        vp.tile([128, 512], bf16, name=f"vtb{tb}", tag=f"vtb{tb}")
                        for tb in range(10)
                    ]
                    qt_g = [
                        qtp.tile([128, GH, 1024], bf16, name=f"qtg{g}", tag=f"qtg{g}")
                        for g in range(NG)
                    ]
                    wo01_sb = wo01p.tile([128, 16, 2, 512], bf16)

                    # ---- phases B+C: projections ----
                    with (
                        tc.sbuf_pool(name="xtf", bufs=1) as xtf,
                        tc.sbuf_pool(name="wvp", bufs=1) as wvp,
                        tc.sbuf_pool(name="wkp", bufs=2) as wkp,
                        tc.sbuf_pool(name="wqp", bufs=3) as wqp,
                        tc.sbuf_pool(name="rtp", bufs=2) as rtp,
                        tc.sbuf_pool(name="rawp", bufs=3) as rawp,
                    ):
                        xt_sb = xtf.tile([128, 16, TX], bf16)
                        wv_sb = wvp.tile([128, 16, 512], bf16)
                        for kc in range(16):
                            nc.sync.dma_start(wv_sb[:, kc, :], wv_t[kc])
                            nc.sync.dma_start(xt_sb[:, kc, :], xt[kc])
                        if rep == 0:
                            nc.sync.dma_start(cos_sb[:], cs_t[0])
                            nc.sync.dma_start(sin_sb[:], cs_t[1])
                            nc.sync.dma_start(ones_sb[:], ones_in[:])
                            nc.sync.dma_start(causal_sb[:], causal[:])
                            for i in range(3):
                                nc.sync.dma_start(m01_sb[i][:], masks[i])
                        nc.sync.dma_start(wo01_sb[:], wo01_t[:])

                        def rope(dst, raw, col0, width):
                            # dst/raw: [128, width]; see module docstring
                            sl = slice(col0, col0 + width)
                            at = rtp.tile([128, 1024], bf16, name="at", tag="at")
                            bt = rtp.tile([128, 1024], bf16, name="bt", tag="bt")
                            nc.vector.tensor_mul(at[:, :width], raw, cos_sb[:, sl])
                            nc.vector.tensor_mul(
                                bt[0:64, :width], raw[64:128, :], sin_sb[64:128, sl]
                            )
                            nc.vector.tensor_mul(
                                bt[64:128, :width], raw[0:64, :], sin_sb[0:64, sl]
                            )
                            nc.vector.tensor_add(dst, at[:, :width], bt[:, :width])

                        # V proj: token tiles 0..7 kc-outer (8 banks), then 8,9
                        with tc.psum_pool(name="vacc", bufs=1) as vacc:
                            pvs = [
                                vacc.tile(
                                    [128, 512], f32, name=f"pv{tb}", tag=f"pv{tb % 8}"
                                )
                                for tb in range(8)
                            ]
                            for kc in range(16):
                                for tb in range(8):
                                    nc.tensor.matmul(
                                        pvs[tb][:],
                                        xt_sb[:, kc, tb * 128 : (tb + 1) * 128],
                                        wv_sb[:, kc, :],
                                        start=(kc == 0),
                                        stop=(kc == 15),
                                    )
                            for tb in range(8):
                                nc.scalar.copy(v_t[tb][:], pvs[tb][:])
                            for tb in range(8, 10):
                                pv = vacc.tile(
                                    [128, 512], f32, name=f"pv{tb}", tag=f"pv{tb % 8}"
                                )
                                for kc in range(16):
                                    nc.tensor.matmul(
                                        pv[:],
                                        xt_sb[:, kc, tb * 128 : (tb + 1) * 128],
                                        wv_sb[:, kc, :],
                                        start=(kc == 0),
                                        stop=(kc == 15),
                                    )
                                nc.scalar.copy(v_t[tb][:], pv[:])

                        with (
                            tc.psum_pool(name="acc", bufs=4) as acc,
                            tc.psum_pool(name="qacc", bufs=2) as qacc,
                        ):
                            for g in range(NG):
                                wk_sb = wkp.tile(
                                    [128, 16, 128], bf16, name=f"wkg{g}", tag="wk"
                                )
                                nc.sync.dma_start(wk_sb[:], wk_t[g])
                                for ts, tw in [(0, 512), (512, 512), (1024, 256)]:
                                    pk = acc.tile(
                                        [128, 512], f32, name=f"pk{g}_{ts}", tag="acc"
                                    )
                                    for kc in range(16):
                                        nc.tensor.matmul(
                                            pk[:, :tw],
                                            wk_sb[:, kc, :],
                                            xt_sb[:, kc, ts : ts + tw],
                                            start=(kc == 0),
                                            stop=(kc == 15),
                                        )
                                    kraw = rawp.tile(
                                        [128, 1024], bf16, name=f"kraw{g}_{ts}",
                                        tag="raw",
                                    )
                                    nc.scalar.copy(kraw[:, :tw], pk[:, :tw])
                                    rope(
                                        kt_g[g][:, ts : ts + tw], kraw[:, :tw], ts, tw
                                    )

                            # phase C: Q proj, wq streamed once
                            for h in range(NH):
                                g, m = divmod(h, GH)
                                wq_sb = wqp.tile(
                                    [128, 16, 128], bf16, name=f"wqh{h}", tag="wq"
                                )
                                nc.sync.dma_start(wq_sb[:], wq_t[h])
                                pq = qacc.tile(
                                    [128, 1024], f32, name=f"pq{h}", tag="pq"
                                )
                                for half in range(2):
                                    for kc in range(16):
                                        nc.tensor.matmul(
                                            pq[:, half * 512 : half * 512 + 512],
                                            wq_sb[:, kc, :],
                                            xt_sb[
                                                :,
                                                kc,
                                                HALO + half * 512 : HALO
                                                + half * 512
                                                + 512,
                                            ],
                                            start=(kc == 0),
                                            stop=(kc == 15),
                                        )
                                qraw = rawp.tile(
                                    [128, 1024], bf16, name=f"qraw{h}", tag="raw"
                                )
                                for half in range(2):
                                    hs = slice(half * 512, half * 512 + 512)
                                    nc.scalar.copy(qraw[:, hs], pq[:, hs])
                                rope(qt_g[g][:, m, :], qraw[:], HALO, 1024)

                    # ---- phase D: attention + interleaved full o-proj ----
                    with (
                        tc.sbuf_pool(name="wo23p", bufs=1) as wo23p,
                        tc.sbuf_pool(name="ytp", bufs=1) as ytp,
                        tc.sbuf_pool(name="etp", bufs=3) as etp,
                        tc.sbuf_pool(name="recp", bufs=2) as recp,
                        tc.sbuf_pool(name="osbp", bufs=2) as osbp,
                        tc.psum_pool(name="stp", bufs=4) as stp,
                        tc.psum_pool(name="smp", bufs=1) as smp,
                        tc.psum_pool(name="otp", bufs=1) as otp,
                        tc.psum_pool(name="posp", bufs=1) as posp,
                    ):
                        wo23_sb = wo23p.tile([128, 16, 2, 512], bf16)
                        nc.sync.dma_start(wo23_sb[:], wo23_t[:])
                        wo_sb = {0: wo01_sb, 1: wo23_sb}

                        yts = {}
                        for c in range(NCHUNK):
                            for g in range(NG):
                                yts[(g, c)] = ytp.tile(
                                    [128, 512], bf16,
                                    name=f"yt{g}_{c}", tag=f"yt{g}_{c}",
                                )

                        pos_live = {}

                        def oproj_part(c, cpair, mlo):
                            # heads mlo..mlo+8 of chunk c into output column
                            # pair cpair (csx 2*cpair, 2*cpair+1)
                            if c < 0:
                                return
                            if mlo == 0:
                                pos_live[(c, cpair)] = [
                                    posp.tile(
                                        [128, 512], f32,
                                        name=f"po{c}_{cpair}_{cx}", tag=f"pos{cx}",
                                    )
                                    for cx in range(2)
                                ]
                            pos = pos_live[(c, cpair)]
                            for m in range(mlo, mlo + 8):
                                for cx in range(2):
                                    nc.tensor.matmul(
                                        pos[cx][:],
                                        yts[(m // 4, c)][
                                            :, (m % 4) * 128 : (m % 4) * 128 + 128
                                        ],
                                        wo_sb[cpair][:, m, cx, :],
                                        start=(m == 0),
                                        stop=(m == 15),
                                    )
                            if mlo == 8:
                                osb = osbp.tile(
                                    [128, 2, 512], f32,
                                    name=f"osb{c}_{cpair}", tag="osb",
                                )
                                for cx in range(2):
                                    nc.scalar.copy(osb[:, cx, :], pos[cx][:])
                                nc.sync.dma_start(
                                    o_out[
                                        c * 128 : c * 128 + 128,
                                        cpair * 1024 : cpair * 1024 + 1024,
                                    ],
                                    osb[:].rearrange("p a b -> p (a b)"),
                                )
                                del pos_live[(c, cpair)]

                        def oproj_quarter(c, quarter):
                            oproj_part(c, quarter // 2, (quarter % 2) * 8)

                        for c in range(NCHUNK):
                            mi = min(c, 2)
                            for g in range(NG):
                                sts = []
                                for kb in range(3):
                                    st = stp.tile(
                                        [128, 512], f32,
                                        name=f"st{c}_{g}_{kb}", tag="st",
                                    )
                                    nc.tensor.matmul(
                                        st[:],
                                        kt_g[g][
                                            :,
                                            c * 128 + kb * 128 : c * 128
                                            + kb * 128
                                            + 128,
                                        ],
                                        qt_g[g][:, :, c * 128 : c * 128 + 128],
                                        start=True,
                                        stop=True,
                                    )
                                    sts.append(st)
                                # PE bubble filler: o-proj quarter of chunk c-1
                                oproj_quarter(c - 1, g)

                                et = etp.tile(
                                    [128, 3, 512], bf16, name=f"et{c}_{g}", tag="et"
                                )
                                for kb in range(3):
                                    nc.scalar.activation(
                                        et[:, kb, :],
                                        sts[kb][:],
                                        exp_t,
                                        bias=0.0,
                                        scale=SCALE,
                                    )
                                et01 = et[:, 0:2, :]
                                nc.vector.tensor_mul(
                                    et01,
                                    et01,
                                    m01_sb[mi][:].rearrange("p (a b) -> p a b", a=2),
                                )
                                nc.vector.tensor_mul(
                                    et[:, 2, :], et[:, 2, :], causal_sb[:]
                                )

                                sums = smp.tile(
                                    [128, 512], f32, name=f"sm{c}_{g}", tag="sm"
                                )
                                for kb in range(3):
                                    nc.tensor.matmul(
                                        sums[:],
                                        ones_sb[:],
                                        et[:, kb, :],
                                        start=(kb == 0),
                                        stop=(kb == 2),
                                    )
                                outt = otp.tile(
                                    [128, 512], f32, name=f"ot{c}_{g}", tag="ot"
                                )
                                for kb in range(3):
                                    nc.tensor.matmul(
                                        outt[:],
                                        v_t[c + kb][:, g * 128 : (g + 1) * 128],
                                        et[:, kb, :],
                                        start=(kb == 0),
                                        stop=(kb == 2),
                                    )
                                rec = recp.tile(
                                    [128, 512], f32, name=f"rc{c}_{g}", tag="rec"
                                )
                                nc.vector.reciprocal(rec[:], sums[:])
                                nc.vector.tensor_mul(
                                    yts[(g, c)][:], outt[:], rec[:]
                                )
                        # tail: o-proj of last chunk
                        for q in range(4):
                            oproj_quarter(NCHUNK - 1, q)

    nc.compile()
    return nc


def _prep_shared(wq, wk, wv, wo, rope_cache):
    """Host-side weight swizzles shared by all cores."""
    perm = np.concatenate([np.arange(0, 128, 2), np.arange(1, 128, 2)])

    wq_p = wq.reshape(NH, 128, C)[:, perm, :]  # [h, d, C]
    wq_sw = np.ascontiguousarray(
        wq_p.reshape(NH, 128, 16, 128).transpose(0, 3, 2, 1)
    ).astype(np.float32)  # [h, p, kc, n]

    wk_p = wk.reshape(NKV, 128, C)[:, perm, :]
    wk_sw = np.ascontiguousarray(
        wk_p.reshape(NKV, 128, 16, 128).transpose(0, 3, 2, 1)
    ).astype(np.float32)

    wv_sw = np.ascontiguousarray(
        wv.reshape(NKV * D, 16, 128).transpose(1, 2, 0)
    ).astype(np.float32)  # [kc, p, n=512]

    # wo given [C, HD]; woT tiles [m, cs, p(d), n(c)] -> resident [p, m, cs, n]
    wo_sw = np.ascontiguousarray(
        wo.T.reshape(16, 128, 4, 512).transpose(0, 2, 1, 3)
    ).astype(np.float32)
    wo01_sw = np.ascontiguousarray(wo_sw[:, 0:2].transpose(2, 0, 1, 3))
    wo23_sw = np.ascontiguousarray(wo_sw[:, 2:4].transpose(2, 0, 1, 3))

    ones = np.ones((128, 128), dtype=np.float32)

    # multiplicative masks (key j on partitions, (head, query i) on free)
    j = np.arange(128)[:, None]
    i = np.arange(128)[None, :]
    causal = np.where(j <= i, 1.0, 0.0).astype(np.float32)
    causal4 = np.tile(causal, (1, 4))  # [128, 512]
    kb0_int = np.where(j > i, 1.0, 0.0).astype(np.float32)
    kb0_int4 = np.tile(kb0_int, (1, 4))
    ones4 = np.ones((128, 512), dtype=np.float32)
    zeros4 = np.zeros((128, 512), dtype=np.float32)

    return wq_sw, wk_sw, wv_sw, wo01_sw, wo23_sw, ones, causal4, kb0_int4, ones4, zeros4


def _make_in_maps(x, wq, wk, wv, wo, rope_cache):
    (wq_sw, wk_sw, wv_sw, wo01_sw, wo23_sw, ones, causal4, kb0_int4, ones4, zeros4) = (
        _prep_shared(wq, wk, wv, wo, rope_cache)
    )
    import ml_dtypes

    b16 = ml_dtypes.bfloat16

    in_maps = []
    for core in range(N_CORES):
        b, tq = divmod(core, 4)
        t0 = tq * TCORE

        # x^T with left halo, zero-padded below t=0; layout [kc, p, t]
        xpad = np.zeros((C, TX), dtype=np.float32)
        lo = t0 - HALO
        src_lo = max(lo, 0)
        xpad[:, src_lo - lo :] = x[b, src_lo : t0 + TCORE, :].T
        xt_sw = np.ascontiguousarray(xpad.reshape(16, 128, TX))

        # cos/sin tiles [2, 128, TX]: cosdup = [cos|cos], sinsig = [+sin|-sin]
        tglob = np.clip(np.arange(lo, t0 + TCORE), 0, T - 1)
        cs = np.empty((2, 128, TX), dtype=np.float32)
        cs[0, 0:64] = rope_cache[tglob, :, 0].T
        cs[0, 64:128] = cs[0, 0:64]
        cs[1, 0:64] = rope_cache[tglob, :, 1].T
        cs[1, 64:128] = -cs[1, 0:64]

        # mask planes [3, 128, 1024] indexed by min(c, 2)
        mk = np.empty((3, 128, 1024), dtype=np.float32)
        for mi in range(3):
            gc = t0 // 128 + mi
            mk[mi, :, 0:512] = kb0_int4 if gc >= 2 else zeros4
            mk[mi, :, 512:1024] = ones4 if gc >= 1 else zeros4

        in_maps.append(
            {
                "xt": xt_sw.astype(b16),
                "wq_t": wq_sw.astype(b16),
                "wk_t": wk_sw.astype(b16),
                "wv_t": wv_sw.astype(b16),
                "wo01_t": wo01_sw.astype(b16),
                "wo23_t": wo23_sw.astype(b16),
                "cs_t": cs.astype(b16),
                "masks": mk.astype(b16),
                "causal": causal4.astype(b16),
                "ones_in": ones.astype(b16),
            }
        )
    return in_maps


def kernel(x, wq, wk, wv, wo, rope_cache):
    x = np.asarray(x, dtype=np.float32)
    wq = np.asarray(wq, dtype=np.float32)
    wk = np.asarray(wk, dtype=np.float32)
    wv = np.asarray(wv, dtype=np.float32)
    wo = np.asarray(wo, dtype=np.float32)
    rope_cache = np.asarray(rope_cache, dtype=np.float32)

    if "nc" not in _CACHE:
        _CACHE["nc"] = _build_nc()
    nc = _CACHE["nc"]

    in_maps = _make_in_maps(x, wq, wk, wv, wo, rope_cache)
    _CACHE["in_maps"] = in_maps

    res = bass_utils.run_bass_kernel_spmd(nc, in_maps, core_ids=list(range(N_CORES)))

    out = np.empty((B, T, C), dtype=np.float32)
    for core in range(N_CORES):
        b, tq = divmod(core, 4)
        out[b, tq * TCORE : (tq + 1) * TCORE, :] = res.results[core]["o_out"]
    return out
